# revision 1
# baseline (speedup 1.0000x reference)
"""Causal attention (flattened-head GQA variant) for TRN2, 8 NeuronCores.

Problem structure exploited:
  - K/V are group-projections tiled 4x along the head dim, and the score
    contraction runs over the full flattened 1024 dim.  Algebraically:
        att = Q @ tile(Kg,4)^T = (sum of Q's four 256-col blocks) @ Kg^T
        out = att_sm @ tile(Vg,4) = tile(att_sm @ Vg, 4)
    so the device only computes with 256-wide Qsum/Kg/Vg.
  - Softmax needs no max-subtraction here (logits bounded ~60; exp fits fp32
    comfortably), so scores are computed directly in the transposed layout
    U^T[s,t] = exp(Kg @ Qsum^T) and fed straight into the AV matmul as the
    stationary operand -- no on-device transposes at all.
  - Row sums come from a ones-column appended to Vg (PSUM col 256).
  - Block-causal skipping: s-tiles entirely above the diagonal are never
    computed; diagonal 128x256 blocks are masked with precomputed 0/1 tiles.
  - Fused chunk loop: for each 256-wide t-chunk, DMA x, project Q/K for the
    chunk, project V for its two s-tiles, then compute score block J=chunk
    (causally needs only chunks <= J) and its AV output.  This fills the
    DMA-paced load phase with score/AV compute and keeps the PE saturated.

Precision: QK path in fp16 (11-bit mantissa, full PE rate, half DMA), scores
accumulated in fp32 PSUM, exp/AV path in bf16 (needs bf16's exponent range:
unnormalized exp values reach ~1e26).  End-to-end absmax rel error vs the
fp32 reference ~5e-3.

Sharding: data-parallel over batch B=8, one batch per core, no collectives.
"""

import os
import numpy as np
import ml_dtypes
from contextlib import ExitStack

import concourse.tile as tile
from concourse import bacc, mybir
from concourse.bass_utils import run_bass_kernel_spmd

B, T, D = 8, 2048, 1024
C = 256          # group width (N_QUERY_GROUPS * HEAD_SIZE)
P = 128
ND = D // P      # 8 contraction tiles for projections
NS = T // P      # 16 s-tiles
JB = 256         # t-chunk width
NJB = T // JB    # 8
NCORES = 8

F32 = mybir.dt.float32
FP16 = mybir.dt.float16
BF16 = mybir.dt.bfloat16


def _build():
    nc = bacc.Bacc("TRN2", target_bir_lowering=False, debug=False)
    xT = nc.dram_tensor("xT", [D, T], FP16, kind="ExternalInput").ap()
    wq = nc.dram_tensor("wq", [D, C], FP16, kind="ExternalInput").ap()
    wk = nc.dram_tensor("wk", [D, C], FP16, kind="ExternalInput").ap()
    wv = nc.dram_tensor("wv", [D, C], FP16, kind="ExternalInput").ap()
    bqk = nc.dram_tensor("bqk", [P, 4], F32, kind="ExternalInput").ap()
    bvb = nc.dram_tensor("bvb", [P, C], FP16, kind="ExternalInput").ap()
    msk = nc.dram_tensor("msk", [P, 2, JB], mybir.dt.float8e4, kind="ExternalInput").ap()
    o = nc.dram_tensor("o", [T, C], F32, kind="ExternalOutput").ap()

    with tile.TileContext(nc) as tc, ExitStack() as ctx:
        cst = ctx.enter_context(tc.tile_pool(name="cst", bufs=1))
        big = ctx.enter_context(tc.tile_pool(name="big", bufs=1))
        up = ctx.enter_context(tc.tile_pool(name="up", bufs=3))
        outp = ctx.enter_context(tc.tile_pool(name="outp", bufs=3))
        pp = ctx.enter_context(tc.tile_pool(name="pp", bufs=2, space="PSUM"))
        pst = ctx.enter_context(tc.tile_pool(name="pst", bufs=4, space="PSUM"))
        pav = ctx.enter_context(tc.tile_pool(name="pav", bufs=2, space="PSUM"))

        bqk_t = cst.tile([P, 4], F32, tag="bqk")
        nc.sync.dma_start(bqk_t[:], bqk)
        bvb_t = cst.tile([P, C], FP16, tag="bvb")
        msk_t = cst.tile([P, 2, JB], mybir.dt.float8e4, tag="msk")

        wr = {}
        for _n in ("q", "k", "v"):
            wr[_n] = cst.tile([P, ND, C], FP16, tag=f"w{_n}", name=f"wr_{_n}")

        # PE warm-up: ~5us of matmuls on a zeroed scratch tile while the
        # first DMAs land -- fills the idle start window and finishes the
        # PE clock ramp (HAM) before real work arrives.
        wrm = cst.tile([P, C], FP16, tag="wrm")
        nc.vector.memset(wrm[:], 0.0)
        for wi in range(16):
            ps_w = pp.tile([P, 2 * JB], F32, tag="pp", name=f"warm_{wi}")
            nc.tensor.matmul(ps_w[:, :C], wrm[:, :P], wrm[:], start=True, stop=True)

        xtr = big.tile([P, ND, T], FP16, tag="xtr")
        qkT = {"q": big.tile([P, 2, T], FP16, tag="qsT", name="qsT"),
               "k": big.tile([P, 2, T], FP16, tag="ksT", name="ksT")}
        vg = big.tile([P, NS, C + 1], BF16, tag="vg")
        nc.vector.memset(vg[:, :, C:C + 1], 8.0)

        uts = {}

        def do_st(J):
            # scores^T -> exp for t-block J
            jt = slice(J * JB, (J + 1) * JB)
            ut = up.tile([P, NS, JB], BF16, tag="ut", name=f"ut_{J}")
            uts[J] = ut
            for sp in range(J + 1):
                si0 = 2 * sp
                ps_t = pst.tile([P, 2 * JB], F32, tag="pst",
                                name=f"pst_{J}_{sp}")
                if sp < J:
                    for h in range(2):
                        si = si0 + h
                        for ct in range(2):
                            nc.tensor.matmul(
                                ps_t[:, h * JB:(h + 1) * JB],
                                qkT["k"][:, ct, si * P:(si + 1) * P],
                                qkT["q"][:, ct, jt],
                                start=(ct == 0), stop=(ct == 1),
                            )
                    nc.scalar.activation(ut[:, si0:si0 + 2, :], ps_t[:],
                                         mybir.ActivationFunctionType.Exp)
                else:
                    # diagonal pair: si0 needs all 256 t-cols; si0+1 only
                    # its second 128 (AV q=0 never reads si0+1) -> N=128
                    for ct in range(2):
                        nc.tensor.matmul(
                            ps_t[:, 0:JB],
                            qkT["k"][:, ct, si0 * P:(si0 + 1) * P],
                            qkT["q"][:, ct, jt],
                            start=(ct == 0), stop=(ct == 1),
                        )
                    for ct in range(2):
                        nc.tensor.matmul(
                            ps_t[:, JB:JB + P],
                            qkT["k"][:, ct, (si0 + 1) * P:(si0 + 2) * P],
                            qkT["q"][:, ct, J * JB + P:(J + 1) * JB],
                            start=(ct == 0), stop=(ct == 1),
                        )
                    nc.scalar.activation(ut[:, si0, :], ps_t[:, 0:JB],
                                         mybir.ActivationFunctionType.Exp)
                    nc.scalar.activation(ut[:, si0 + 1, P:JB],
                                         ps_t[:, JB:JB + P],
                                         mybir.ActivationFunctionType.Exp)
                    nc.vector.tensor_tensor(ut[:, si0, :], ut[:, si0, :],
                                            msk_t[:, 0, :],
                                            mybir.AluOpType.mult)
                    nc.vector.tensor_tensor(ut[:, si0 + 1, P:JB],
                                            ut[:, si0 + 1, P:JB],
                                            msk_t[:, 1, P:JB],
                                            mybir.AluOpType.mult)

        def do_v(tb):
            # V projection for chunk tb's two s-tiles
            for si in (2 * tb, 2 * tb + 1):
                pv = pav.tile([P, C + 1], F32, tag="pav", name=f"pv_{si}")[:, :JB]
                for d in range(ND):
                    nc.tensor.matmul(
                        pv,
                        xtr[:, d, si * P:(si + 1) * P],
                        wr["v"][:, d, :],
                        start=(d == 0), stop=(d == ND - 1),
                    )
                nc.vector.tensor_tensor(vg[:, si, :C], pv, bvb_t[:],
                                        mybir.AluOpType.add)

        def do_av(J):
            ut = uts[J]
            for q in range(2):
                tci = 2 * J + q
                pa = pav.tile([P, C + 1], F32, tag="pav", name=f"pav_{J}_{q}")
                for si in range(tci + 1):
                    nc.tensor.matmul(
                        pa[:],
                        ut[:, si, q * P:(q + 1) * P],
                        vg[:, si, :],
                        start=(si == 0), stop=(si == tci),
                    )
                recip = outp.tile([P, 1], F32, tag="recip")
                nc.vector.reciprocal(recip[:], pa[:, C:C + 1])
                ob = outp.tile([P, C], F32, tag="ob")
                nc.vector.tensor_scalar_mul(ob[:], pa[:, :C], recip[:])
                nc.sync.dma_start(o[tci * P:(tci + 1) * P, :], ob[:])

        def do_proj(psl):
            pw = psl.stop - psl.start
            for mi, mat in enumerate(("q", "k")):
                for ct in range(2):
                    ps_p = pp.tile([P, 2 * JB], F32, tag="pp",
                                   name=f"pp_{mat}{ct}_{psl.start}")[:, :pw]
                    for d in range(ND):
                        nc.tensor.matmul(
                            ps_p,
                            wr[mat][:, d, ct * P:(ct + 1) * P],
                            xtr[:, d, psl],
                            start=(d == 0), stop=(d == ND - 1),
                        )
                    nc.vector.tensor_scalar_add(
                        qkT[mat][:, ct, psl],
                        ps_p,
                        bqk_t[:, 2 * mi + ct: 2 * mi + ct + 1],
                    )

        # ---- pair 0: startup DMAs, projections, scores only (V/AV for
        # chunks 0-1 are deferred into pair 1 so the DMA queue delivers
        # x chunks 2-3 before the V weights) ----
        ts0 = slice(0, JB)
        xsrc0 = xT[:, ts0].rearrange("(o p) t -> p o t", p=P)
        wq_src = wq.rearrange("(o p) c -> p o c", p=P)
        nc.sync.dma_start(wr["q"][:, :, 0:P], wq_src[:, :, 0:P])
        nc.sync.dma_start(xtr[:, 0:2, ts0], xsrc0[:, 0:2, :])
        nc.sync.dma_start(xtr[:, 2:4, ts0], xsrc0[:, 2:4, :])
        nc.sync.dma_start(wr["q"][:, :, P:C], wq_src[:, :, P:C])
        nc.sync.dma_start(xtr[:, 4:6, ts0], xsrc0[:, 4:6, :])
        nc.sync.dma_start(xtr[:, 6:8, ts0], xsrc0[:, 6:8, :])
        nc.sync.dma_start(wr["k"][:], wk.rearrange("(o p) c -> p o c", p=P))
        ts1 = slice(JB, 2 * JB)
        nc.sync.dma_start(xtr[:, :, ts1],
                          xT[:, ts1].rearrange("(o p) t -> p o t", p=P))
        nc.sync.dma_start(msk_t[:], msk)
        nc.sync.dma_start(bvb_t[:], bvb)
        do_proj(ts0)
        do_proj(ts1)
        do_st(0)
        do_st(1)

        # ---- pair 1: x chunks 2-3 queue ahead of the V weights; then the
        # deferred V/AV for chunks 0-1 (emitted after the wv DMA so the RAW
        # dependency is tracked and properly semaphore-guarded) ----
        pts = slice(2 * JB, 4 * JB)
        nc.sync.dma_start(xtr[:, :, pts],
                          xT[:, pts].rearrange("(o p) t -> p o t", p=P))
        nc.sync.dma_start(wr["v"][:], wv.rearrange("(o p) c -> p o c", p=P))
        do_v(0)
        do_av(0)
        do_v(1)
        do_av(1)
        do_proj(pts)
        for tb in (2, 3):
            do_st(tb)
            do_v(tb)
            do_av(tb)

        # ---- pairs 2-3 ----
        for pb in (2, 3):
            tb0 = 2 * pb
            pts = slice(tb0 * JB, (tb0 + 2) * JB)
            nc.sync.dma_start(xtr[:, :, pts],
                              xT[:, pts].rearrange("(o p) t -> p o t", p=P))
            do_proj(pts)
            for tb in (tb0, tb0 + 1):
                do_st(tb)
                do_v(tb)
                do_av(tb)

    nc.compile()
    return nc


_CACHE = {}
LAST_EXEC_TIME_NS = None


def _get_nc():
    if "nc" not in _CACHE:
        _CACHE["nc"] = _build()
    return _CACHE["nc"]


def kernel(x, Wq, bq, Wk, bk, Wv, bv):
    x = np.asarray(x, dtype=np.float32)
    Wq = np.asarray(Wq, dtype=np.float32)
    bq = np.asarray(bq, dtype=np.float32)
    Wk = np.asarray(Wk, dtype=np.float32)
    bk = np.asarray(bk, dtype=np.float32)
    Wv = np.asarray(Wv, dtype=np.float32)
    bv = np.asarray(bv, dtype=np.float32)

    # Fold the 4x head-tiling into the weights: contraction with tile(Kg,4)
    # equals contraction of block-summed Q with Kg.
    wq_s = Wq.reshape(D, 4, C).sum(axis=1, dtype=np.float64).astype(np.float32)
    bq_s = bq.reshape(4, C).sum(axis=0, dtype=np.float64).astype(np.float32)

    bqk = np.stack([bq_s[:P], bq_s[P:], bk[:P], bk[P:]], axis=1).astype(np.float32)
    bvb = np.broadcast_to(bv, (P, C)).astype(np.float32)

    # Diagonal-block causal masks: keep t >= s  <=>  j >= 128*m + p.
    jj = np.arange(JB)[None, None, :]
    pp_ = np.arange(P)[:, None, None]
    mm = np.arange(2)[None, :, None]
    msk = (jj >= P * mm + pp_).astype(ml_dtypes.float8_e4m3)

    shared = {
        "wq": np.ascontiguousarray(wq_s.astype(np.float16)),
        "wk": np.ascontiguousarray(Wk.astype(np.float16)),
        "wv": np.ascontiguousarray(Wv.astype(np.float16)),
        "bqk": np.ascontiguousarray(bqk), "bvb": np.ascontiguousarray(bvb.astype(np.float16)),
        "msk": np.ascontiguousarray(msk),
    }
    in_maps = []
    for b in range(B):
        m = dict(shared)
        m["xT"] = np.ascontiguousarray(x[b].T.astype(np.float16))
        in_maps.append(m)

    nc = _get_nc()
    try:
        res = run_bass_kernel_spmd(nc, in_maps, core_ids=list(range(NCORES)))
    except ModuleNotFoundError:
        # BASS_TRACE=1 requests NTFF profiling, but this container type has
        # no axon NTFF hook (antenv.axon_hooks absent) -- rerun untraced.
        os.environ["BASS_NEVER_TRACE"] = "1"
        res = run_bass_kernel_spmd(nc, in_maps, core_ids=list(range(NCORES)))
    global LAST_EXEC_TIME_NS
    LAST_EXEC_TIME_NS = res.exec_time_ns
    if res.exec_time_ns is not None:
        print(f"HW exec time: {res.exec_time_ns} ns")

    out = np.empty((1, B, T, 4 * C), dtype=np.float32)
    for b in range(B):
        ob = res.results[b]["o"]
        out[0, b] = np.tile(ob, (1, 4))
    return out



# revision 27
# speedup vs baseline: 1.0288x; 1.0288x over previous
"""Causal attention (flattened-head GQA variant) for TRN2, 8 NeuronCores.

Problem structure exploited:
  - K/V are group-projections tiled 4x along the head dim, and the score
    contraction runs over the full flattened 1024 dim.  Algebraically:
        att = Q @ tile(Kg,4)^T = (sum of Q's four 256-col blocks) @ Kg^T
        out = att_sm @ tile(Vg,4) = tile(att_sm @ Vg, 4)
    so the device only computes with 256-wide Qsum/Kg/Vg.
  - Softmax needs no max-subtraction here (logits bounded ~60; exp fits fp32
    comfortably), so scores are computed directly in the transposed layout
    U^T[s,t] = exp(Kg @ Qsum^T) and fed straight into the AV matmul as the
    stationary operand -- no on-device transposes at all.
  - Row sums come from a ones-column appended to Vg (PSUM col 256).
  - Block-causal skipping: s-tiles entirely above the diagonal are never
    computed; diagonal 128x256 blocks are masked with precomputed 0/1 tiles.
  - Fused chunk loop: for each 256-wide t-chunk, DMA x, project Q/K for the
    chunk, project V for its two s-tiles, then compute score block J=chunk
    (causally needs only chunks <= J) and its AV output.  This fills the
    DMA-paced load phase with score/AV compute and keeps the PE saturated.

Precision: QK path in fp16 (11-bit mantissa, full PE rate, half DMA), scores
accumulated in fp32 PSUM, exp/AV path in bf16 (needs bf16's exponent range:
unnormalized exp values reach ~1e26).  End-to-end absmax rel error vs the
fp32 reference ~5e-3.

Sharding: data-parallel over batch B=8, one batch per core, no collectives.
"""

import os
import numpy as np
import ml_dtypes
from contextlib import ExitStack

import concourse.tile as tile
from concourse import bacc, mybir
from concourse.bass_utils import run_bass_kernel_spmd

B, T, D = 8, 2048, 1024
C = 256          # group width (N_QUERY_GROUPS * HEAD_SIZE)
P = 128
ND = D // P      # 8 contraction tiles for projections
NS = T // P      # 16 s-tiles
JB = 256         # t-chunk width
NJB = T // JB    # 8
NCORES = 8
NWARM = 1        # warm-up matmuls (256 cols each) before first real work
NPAD1 = 0        # PE filler between q-proj d0-3 and d4-7 (x arrival gap)
NPAD2 = 0        # PE filler covering the qk bias-add (DVE) latency

F32 = mybir.dt.float32
FP16 = mybir.dt.float16
BF16 = mybir.dt.bfloat16


def _build():
    nc = bacc.Bacc("TRN2", target_bir_lowering=False, debug=False)
    xT = nc.dram_tensor("xT", [D, T], FP16, kind="ExternalInput").ap()
    wq = nc.dram_tensor("wq", [D, C], FP16, kind="ExternalInput").ap()
    wk = nc.dram_tensor("wk", [D, C], FP16, kind="ExternalInput").ap()
    wv = nc.dram_tensor("wv", [D, C], FP16, kind="ExternalInput").ap()
    bqk = nc.dram_tensor("bqk", [P, 4], F32, kind="ExternalInput").ap()
    bvb = nc.dram_tensor("bvb", [P, C], FP16, kind="ExternalInput").ap()
    msk = nc.dram_tensor("msk", [P, 2, JB], mybir.dt.float8e4, kind="ExternalInput").ap()
    o = nc.dram_tensor("o", [T, C], FP16, kind="ExternalOutput").ap()

    with tile.TileContext(nc) as tc, ExitStack() as ctx:
        cst = ctx.enter_context(tc.tile_pool(name="cst", bufs=1))
        big = ctx.enter_context(tc.tile_pool(name="big", bufs=1))
        up = ctx.enter_context(tc.tile_pool(name="up", bufs=3))
        outp = ctx.enter_context(tc.tile_pool(name="outp", bufs=3))
        pp = ctx.enter_context(tc.tile_pool(name="pp", bufs=2, space="PSUM"))
        pst = ctx.enter_context(tc.tile_pool(name="pst", bufs=4, space="PSUM"))
        pav = ctx.enter_context(tc.tile_pool(name="pav", bufs=2, space="PSUM"))

        bqk_t = cst.tile([P, 4], F32, tag="bqk")
        bvb_t = cst.tile([P, C], FP16, tag="bvb")
        msk_t = cst.tile([P, 2, JB], mybir.dt.float8e4, tag="msk")

        wr = {}
        for _n in ("q", "k", "v"):
            wr[_n] = cst.tile([P, ND, C], FP16, tag=f"w{_n}", name=f"wr_{_n}")

        # PE warm-up scratch: memset on the Pool engine, which is idle and
        # issues within ~100ns (DVE's first slot is ~0.8us in), so warm-up
        # matmuls start almost immediately, fill the DMA pipe-fill window,
        # and complete the PE p-state ramp (full clock 3us after first busy).
        wrm = cst.tile([P, C], FP16, tag="wrm")
        nc.gpsimd.memset(wrm[:], 0.0)
        for wi in range(NWARM):
            ps_w = pp.tile([P, 2 * JB], F32, tag="pp", name=f"warm_{wi}")
            nc.tensor.matmul(ps_w[:, :C], wrm[:, :P], wrm[:], start=True, stop=True)

        _padn = [0]

        def pad(n):
            # PE filler matmuls (dep-free) to bridge known DMA-arrival gaps;
            # allocated from pav, which is idle until the first AV.
            for _ in range(n):
                _padn[0] += 1
                ps_f = pav.tile([P, C + 1], F32, tag="pav",
                                name=f"padmm_{_padn[0]}")
                nc.tensor.matmul(ps_f[:, :C], wrm[:, :P], wrm[:],
                                 start=True, stop=True)

        xtr = big.tile([P, ND, T], FP16, tag="xtr")
        qkT = {"q": big.tile([P, 2, T], FP16, tag="qsT", name="qsT"),
               "k": big.tile([P, 2, T], FP16, tag="ksT", name="ksT")}
        vg = big.tile([P, NS, C + 1], BF16, tag="vg")
        nc.vector.memset(vg[:, :, C:C + 1], 8.0)

        uts = {}
        pending_masks = []

        def flush_masks():
            while pending_masks:
                ut, si0 = pending_masks.pop(0)
                nc.vector.tensor_tensor(ut[:, si0, :], ut[:, si0, :],
                                        msk_t[:, 0, :],
                                        mybir.AluOpType.mult)
                nc.vector.tensor_tensor(ut[:, si0 + 1, P:JB],
                                        ut[:, si0 + 1, P:JB],
                                        msk_t[:, 1, P:JB],
                                        mybir.AluOpType.mult)

        def do_st(J):
            # scores^T -> exp for t-block J
            jt = slice(J * JB, (J + 1) * JB)
            ut = up.tile([P, NS, JB], BF16, tag="ut", name=f"ut_{J}")
            uts[J] = ut
            for sp in range(J + 1):
                si0 = 2 * sp
                ps_t = pst.tile([P, 2 * JB], F32, tag="pst",
                                name=f"pst_{J}_{sp}")
                if sp < J:
                    for h in range(2):
                        si = si0 + h
                        for ct in range(2):
                            nc.tensor.matmul(
                                ps_t[:, h * JB:(h + 1) * JB],
                                qkT["k"][:, ct, si * P:(si + 1) * P],
                                qkT["q"][:, ct, jt],
                                start=(ct == 0), stop=(ct == 1),
                            )
                    nc.scalar.activation(ut[:, si0:si0 + 2, :], ps_t[:],
                                         mybir.ActivationFunctionType.Exp)
                else:
                    # diagonal pair: si0 needs all 256 t-cols; si0+1 only
                    # its second 128 (AV q=0 never reads si0+1) -> N=128
                    for ct in range(2):
                        nc.tensor.matmul(
                            ps_t[:, 0:JB],
                            qkT["k"][:, ct, si0 * P:(si0 + 1) * P],
                            qkT["q"][:, ct, jt],
                            start=(ct == 0), stop=(ct == 1),
                        )
                    for ct in range(2):
                        nc.tensor.matmul(
                            ps_t[:, JB:JB + P],
                            qkT["k"][:, ct, (si0 + 1) * P:(si0 + 2) * P],
                            qkT["q"][:, ct, J * JB + P:(J + 1) * JB],
                            start=(ct == 0), stop=(ct == 1),
                        )
                    nc.scalar.activation(ut[:, si0, :], ps_t[:, 0:JB],
                                         mybir.ActivationFunctionType.Exp)
                    nc.scalar.activation(ut[:, si0 + 1, P:JB],
                                         ps_t[:, JB:JB + P],
                                         mybir.ActivationFunctionType.Exp)
                    # defer the DVE mask-mults until after do_v's vg-adds:
                    # the masks only gate AV(J) (emitted a stage later, >3us
                    # of slack) while the vg-adds free pav PSUM slots (tight)
                    pending_masks.append((ut, si0))

        def do_v(tb):
            # V projection for chunk tb's two s-tiles
            for si in (2 * tb, 2 * tb + 1):
                pv = pav.tile([P, C + 1], F32, tag="pav", name=f"pv_{si}")[:, :JB]
                for d in range(ND):
                    nc.tensor.matmul(
                        pv,
                        xtr[:, d, si * P:(si + 1) * P],
                        wr["v"][:, d, :],
                        start=(d == 0), stop=(d == ND - 1),
                    )
                nc.vector.tensor_tensor(vg[:, si, :C], pv, bvb_t[:],
                                        mybir.AluOpType.add)
            flush_masks()

        def do_av(J):
            ut = uts[J]
            for q in range(2):
                tci = 2 * J + q
                pa = pav.tile([P, C + 1], F32, tag="pav", name=f"pav_{J}_{q}")
                if not (J == NJB - 1 and q == 1):
                    for si in range(tci + 1):
                        nc.tensor.matmul(
                            pa[:],
                            ut[:, si, q * P:(q + 1) * P],
                            vg[:, si, :],
                            start=(si == 0), stop=(si == tci),
                        )
                    recip = outp.tile([P, 1], F32, tag="recip")
                    nc.vector.reciprocal(recip[:], pa[:, C:C + 1])
                    ob = outp.tile([P, C], FP16, tag="ob")
                    nc.vector.tensor_scalar_mul(ob[:], pa[:, :C], recip[:])
                    # Pool/SWDGE path keeps HWDGE + the SP sequencer free
                    nc.gpsimd.dma_start(o[tci * P:(tci + 1) * P, :], ob[:])
                else:
                    # final output tile: column-split accumulation so the
                    # upper half's recip/scale/DMA chain overlaps the lower
                    # half's matmuls, shortening the end-of-kernel chain.
                    # Upper half includes the ones-column (rowsum -> recip).
                    for si in range(tci + 1):
                        nc.tensor.matmul(
                            pa[:, P:],
                            ut[:, si, q * P:(q + 1) * P],
                            vg[:, si, P:],
                            start=(si == 0), stop=(si == tci),
                        )
                    recip = outp.tile([P, 1], F32, tag="recip")
                    nc.vector.reciprocal(recip[:], pa[:, C:C + 1])
                    ob = outp.tile([P, C], FP16, tag="ob")
                    nc.vector.tensor_scalar_mul(ob[:, P:], pa[:, P:C], recip[:])
                    nc.sync.dma_start(o[tci * P:(tci + 1) * P, P:], ob[:, P:])
                    # separate PSUM tile: tile-granular PSUM deps would
                    # otherwise serialize these matmuls behind A's readers
                    pa_b = pav.tile([P, C + 1], F32, tag="pav",
                                    name=f"pav_{J}_{q}b")
                    for si in range(tci + 1):
                        nc.tensor.matmul(
                            pa_b[:, :P],
                            ut[:, si, q * P:(q + 1) * P],
                            vg[:, si, :P],
                            start=(si == 0), stop=(si == tci),
                        )
                    nc.vector.tensor_scalar_mul(ob[:, :P], pa_b[:, :P], recip[:])
                    nc.sync.dma_start(o[tci * P:(tci + 1) * P, :P], ob[:, :P])

        def do_proj(psl):
            pw = psl.stop - psl.start
            for mi, mat in enumerate(("q", "k")):
                for ct in range(2):
                    ps_p = pp.tile([P, 2 * JB], F32, tag="pp",
                                   name=f"pp_{mat}{ct}_{psl.start}")[:, :pw]
                    for d in range(ND):
                        nc.tensor.matmul(
                            ps_p,
                            wr[mat][:, d, ct * P:(ct + 1) * P],
                            xtr[:, d, psl],
                            start=(d == 0), stop=(d == ND - 1),
                        )
                    # bias-add on the Activation engine (per-partition bias):
                    # keeps the DVE queue short so vg-adds aren't delayed.
                    nc.scalar.activation(
                        qkT[mat][:, ct, psl], ps_p,
                        mybir.ActivationFunctionType.Identity,
                        bias=bqk_t[:, 2 * mi + ct: 2 * mi + ct + 1],
                    )

        # ---- startup: d-granular interleaved gate DMAs.  st0 is gated on
        # wq+wk+x[ts0] (1.5MB); the serial DMA pipe paces everything, so the
        # order below matches PE consumption order (q-proj d0-3, q d4-7,
        # k d0-3, k d4-7), with the tiny bias/mask DMAs deferred past the
        # gate and x[ts1]/wv/x[ts23] queued just-in-time behind it. ----
        ts0 = slice(0, JB)
        ts1 = slice(JB, 2 * JB)
        pts23 = slice(2 * JB, 4 * JB)
        xsrc = xT.rearrange("(o p) t -> p o t", p=P)
        wq_src = wq.rearrange("(o p) c -> p o c", p=P)
        wk_src = wk.rearrange("(o p) c -> p o c", p=P)
        wv_src = wv.rearrange("(o p) c -> p o c", p=P)
        nc.sync.dma_start(wr["q"][:, 0:4, :], wq_src[:, 0:4, :])
        nc.sync.dma_start(xtr[:, 0:4, ts0], xsrc[:, 0:4, ts0])
        nc.sync.dma_start(wr["q"][:, 4:8, :], wq_src[:, 4:8, :])
        nc.sync.dma_start(xtr[:, 4:8, ts0], xsrc[:, 4:8, ts0])
        nc.sync.dma_start(wr["k"][:, 0:4, :], wk_src[:, 0:4, :])
        nc.sync.dma_start(wr["k"][:, 4:8, :], wk_src[:, 4:8, :])
        nc.sync.dma_start(bqk_t[:], bqk)
        nc.sync.dma_start(xtr[:, :, ts1], xsrc[:, :, ts1])
        nc.sync.dma_start(msk_t[:], msk)
        nc.sync.dma_start(bvb_t[:], bvb)
        nc.sync.dma_start(wr["v"][:], wv_src)
        nc.sync.dma_start(xtr[:, :, pts23], xsrc[:, :, pts23])
        pts45 = slice(4 * JB, 6 * JB)
        pts67 = slice(6 * JB, 8 * JB)
        nc.sync.dma_start(xtr[:, :, pts45], xsrc[:, :, pts45])
        nc.sync.dma_start(xtr[:, :, pts67], xsrc[:, :, pts67])

        # First projection (chunk 0) d-outer with 4 parallel PSUM
        # accumulators, so each arriving (w, x) d-group unlocks work.
        psq = [pp.tile([P, 2 * JB], F32, tag="pp", name=f"psq{ct}")[:, :JB]
               for ct in range(2)]
        psk = [pst.tile([P, 2 * JB], F32, tag="pst", name=f"psk{ct}")[:, :JB]
               for ct in range(2)]

        def proj0_mm(mat, accs, ds):
            for d in ds:
                for ct in range(2):
                    nc.tensor.matmul(
                        accs[ct], wr[mat][:, d, ct * P:(ct + 1) * P],
                        xtr[:, d, ts0], start=(d == 0), stop=(d == ND - 1))

        proj0_mm("q", psq, range(0, 4))
        pad(NPAD1)
        proj0_mm("q", psq, range(4, 8))
        proj0_mm("k", psk, range(0, 4))
        proj0_mm("k", psk, range(4, 8))
        # startup biases stay on DVE: st0 is latency-critical and the DVE
        # queue is empty here (Act is slower per-op).  Queue order q0, k0
        # first: st0's opening ct0 matmul needs exactly those two.
        for mi, ct in ((0, 0), (1, 0), (0, 1), (1, 1)):
            mat, accs = (("q", psq), ("k", psk))[mi]
            nc.vector.tensor_scalar_add(
                qkT[mat][:, ct, ts0], accs[ct],
                bqk_t[:, 2 * mi + ct: 2 * mi + ct + 1])
        pad(NPAD2)
        # Software-pipelined chunk loop: each AV(t) is emitted one stage
        # after its st(t)/V(t), so the DVE mask/bias chains and Act exps
        # that gate AV's final si-matmuls always have >1.5us of PE runway
        # and never stall the tensor engine.
        do_st(0)
        do_proj(ts1)
        do_st(1)
        do_v(0)
        do_v(1)
        do_av(0)
        do_proj(pts23)
        do_av(1)
        do_st(2)
        do_v(2)
        do_st(3)
        do_v(3)
        do_av(2)
        do_proj(pts45)
        do_av(3)
        do_st(4)
        do_v(4)
        do_st(5)
        do_v(5)
        do_av(4)
        do_proj(pts67)
        do_av(5)
        do_st(6)
        do_v(6)
        do_st(7)
        do_v(7)
        do_av(6)
        do_av(7)

    nc.compile()
    return nc


_CACHE = {}
LAST_EXEC_TIME_NS = None


def _get_nc():
    if "nc" not in _CACHE:
        _CACHE["nc"] = _build()
    return _CACHE["nc"]


def kernel(x, Wq, bq, Wk, bk, Wv, bv):
    x = np.asarray(x, dtype=np.float32)
    Wq = np.asarray(Wq, dtype=np.float32)
    bq = np.asarray(bq, dtype=np.float32)
    Wk = np.asarray(Wk, dtype=np.float32)
    bk = np.asarray(bk, dtype=np.float32)
    Wv = np.asarray(Wv, dtype=np.float32)
    bv = np.asarray(bv, dtype=np.float32)

    # Fold the 4x head-tiling into the weights: contraction with tile(Kg,4)
    # equals contraction of block-summed Q with Kg.
    wq_s = Wq.reshape(D, 4, C).sum(axis=1, dtype=np.float64).astype(np.float32)
    bq_s = bq.reshape(4, C).sum(axis=0, dtype=np.float64).astype(np.float32)

    bqk = np.stack([bq_s[:P], bq_s[P:], bk[:P], bk[P:]], axis=1).astype(np.float32)
    bvb = np.broadcast_to(bv, (P, C)).astype(np.float32)

    # Diagonal-block causal masks: keep t >= s  <=>  j >= 128*m + p.
    jj = np.arange(JB)[None, None, :]
    pp_ = np.arange(P)[:, None, None]
    mm = np.arange(2)[None, :, None]
    msk = (jj >= P * mm + pp_).astype(ml_dtypes.float8_e4m3)

    shared = {
        "wq": np.ascontiguousarray(wq_s.astype(np.float16)),
        "wk": np.ascontiguousarray(Wk.astype(np.float16)),
        "wv": np.ascontiguousarray(Wv.astype(np.float16)),
        "bqk": np.ascontiguousarray(bqk), "bvb": np.ascontiguousarray(bvb.astype(np.float16)),
        "msk": np.ascontiguousarray(msk),
    }
    in_maps = []
    for b in range(B):
        m = dict(shared)
        m["xT"] = np.ascontiguousarray(x[b].T.astype(np.float16))
        in_maps.append(m)

    nc = _get_nc()
    try:
        res = run_bass_kernel_spmd(nc, in_maps, core_ids=list(range(NCORES)))
    except ModuleNotFoundError:
        # BASS_TRACE=1 requests NTFF profiling, but this container type has
        # no axon NTFF hook (antenv.axon_hooks absent) -- rerun untraced.
        os.environ["BASS_NEVER_TRACE"] = "1"
        res = run_bass_kernel_spmd(nc, in_maps, core_ids=list(range(NCORES)))
    global LAST_EXEC_TIME_NS
    LAST_EXEC_TIME_NS = res.exec_time_ns
    if res.exec_time_ns is not None:
        print(f"HW exec time: {res.exec_time_ns} ns")

    out = np.empty((1, B, T, 4 * C), dtype=np.float32)
    for b in range(B):
        ob = np.asarray(res.results[b]["o"], dtype=np.float32)
        out[0, b] = np.tile(ob, (1, 4))
    return out



# revision 53
# speedup vs baseline: 1.0373x; 1.0083x over previous
"""Causal attention (flattened-head GQA variant) for TRN2, 8 NeuronCores.

Problem structure exploited:
  - K/V are group-projections tiled 4x along the head dim, and the score
    contraction runs over the full flattened 1024 dim.  Algebraically:
        att = Q @ tile(Kg,4)^T = (sum of Q's four 256-col blocks) @ Kg^T
        out = att_sm @ tile(Vg,4) = tile(att_sm @ Vg, 4)
    so the device only computes with 256-wide Qsum/Kg/Vg.
  - Softmax needs no max-subtraction here (logits bounded ~60; exp fits fp32
    comfortably), so scores are computed directly in the transposed layout
    U^T[s,t] = exp(Kg @ Qsum^T) and fed straight into the AV matmul as the
    stationary operand -- no on-device transposes at all.
  - Row sums come from a ones-column appended to Vg (PSUM col 256).
  - Block-causal skipping: s-tiles entirely above the diagonal are never
    computed; diagonal 128x256 blocks are masked with precomputed 0/1 tiles.
  - Fused chunk loop: for each 256-wide t-chunk, DMA x, project Q/K for the
    chunk, project V for its two s-tiles, then compute score block J=chunk
    (causally needs only chunks <= J) and its AV output.  This fills the
    DMA-paced load phase with score/AV compute and keeps the PE saturated.

Precision: QK path in fp16 (11-bit mantissa, full PE rate, half DMA), scores
accumulated in fp32 PSUM, exp/AV path in bf16 (needs bf16's exponent range:
unnormalized exp values reach ~1e26).  End-to-end absmax rel error vs the
fp32 reference ~5e-3.

Sharding: data-parallel over batch B=8, one batch per core, no collectives.
"""

import os
import numpy as np
import ml_dtypes
from contextlib import ExitStack

import concourse.tile as tile
from concourse import bacc, mybir
from concourse.bass_utils import run_bass_kernel_spmd

B, T, D = 8, 2048, 1024
C = 256          # group width (N_QUERY_GROUPS * HEAD_SIZE)
P = 128
ND = D // P      # 8 contraction tiles for projections
NS = T // P      # 16 s-tiles
JB = 256         # t-chunk width
NJB = T // JB    # 8
NCORES = 8
NWARM = 1        # one PE touch latches pe_busy_start for the p-state ramp
NPAD1 = 0        # PE filler between q-proj d0-3 and d4-7 (x arrival gap)
NPAD2 = 0        # PE filler covering the qk bias-add (DVE) latency

F32 = mybir.dt.float32
FP16 = mybir.dt.float16
BF16 = mybir.dt.bfloat16


def _build():
    nc = bacc.Bacc("TRN2", target_bir_lowering=False, debug=False)
    xT = nc.dram_tensor("xT", [D, T], FP16, kind="ExternalInput").ap()
    wq = nc.dram_tensor("wq", [D, C], FP16, kind="ExternalInput").ap()
    wk = nc.dram_tensor("wk", [D, C], FP16, kind="ExternalInput").ap()
    wv = nc.dram_tensor("wv", [D, C], FP16, kind="ExternalInput").ap()
    bqk = nc.dram_tensor("bqk", [P, 4], F32, kind="ExternalInput").ap()
    bvb = nc.dram_tensor("bvb", [P, C], FP16, kind="ExternalInput").ap()
    msk = nc.dram_tensor("msk", [P, 2, JB], mybir.dt.float8e4, kind="ExternalInput").ap()
    o = nc.dram_tensor("o", [T, C], FP16, kind="ExternalOutput").ap()

    with tile.TileContext(nc) as tc, ExitStack() as ctx:
        cst = ctx.enter_context(tc.tile_pool(name="cst", bufs=1))
        big = ctx.enter_context(tc.tile_pool(name="big", bufs=1))
        up = ctx.enter_context(tc.tile_pool(name="up", bufs=3))
        outp = ctx.enter_context(tc.tile_pool(name="outp", bufs=3))
        pp = ctx.enter_context(tc.tile_pool(name="pp", bufs=2, space="PSUM"))
        pst = ctx.enter_context(tc.tile_pool(name="pst", bufs=4, space="PSUM"))
        pav = ctx.enter_context(tc.tile_pool(name="pav", bufs=2, space="PSUM"))

        bqk_t = cst.tile([P, 4], F32, tag="bqk")
        bvb_t = cst.tile([P, C], FP16, tag="bvb")
        msk_t = cst.tile([P, 2, JB], mybir.dt.float8e4, tag="msk")

        wr = {}
        for _n in ("q", "k", "v"):
            wr[_n] = cst.tile([P, ND, C], FP16, tag=f"w{_n}", name=f"wr_{_n}")

        # PE warm-up scratch: memset on the Pool engine, which is idle and
        # issues within ~100ns (DVE's first slot is ~0.8us in), so warm-up
        # matmuls start almost immediately, fill the DMA pipe-fill window,
        # and complete the PE p-state ramp (full clock 3us after first busy).
        wrm = cst.tile([P, C], FP16, tag="wrm")
        nc.gpsimd.memset(wrm[:], 0.0)
        for wi in range(NWARM):
            ps_w = pp.tile([P, 2 * JB], F32, tag="pp", name=f"warm_{wi}")
            nc.tensor.matmul(ps_w[:, :C], wrm[:, :P], wrm[:], start=True, stop=True)

        _padn = [0]

        def pad(n):
            # PE filler matmuls (dep-free) to bridge known DMA-arrival gaps;
            # allocated from pav, which is idle until the first AV.
            for _ in range(n):
                _padn[0] += 1
                ps_f = pav.tile([P, C + 1], F32, tag="pav",
                                name=f"padmm_{_padn[0]}")
                nc.tensor.matmul(ps_f[:, :C], wrm[:, :P], wrm[:],
                                 start=True, stop=True)

        xtr = big.tile([P, ND, T], FP16, tag="xtr")
        qkT = {"q": big.tile([P, 2, T], FP16, tag="qsT", name="qsT"),
               "k": big.tile([P, 2, T], FP16, tag="ksT", name="ksT")}
        vg = big.tile([P, NS, C + 1], BF16, tag="vg")
        nc.vector.memset(vg[:, :, C:C + 1], 8.0)

        uts = {}
        pending_masks = []

        def flush_masks():
            while pending_masks:
                ut, si0 = pending_masks.pop(0)
                nc.vector.tensor_tensor(ut[:, si0, :], ut[:, si0, :],
                                        msk_t[:, 0, :],
                                        mybir.AluOpType.mult)
                nc.vector.tensor_tensor(ut[:, si0 + 1, P:JB],
                                        ut[:, si0 + 1, P:JB],
                                        msk_t[:, 1, P:JB],
                                        mybir.AluOpType.mult)

        def do_st(J):
            # scores^T -> exp for t-block J
            jt = slice(J * JB, (J + 1) * JB)
            ut = up.tile([P, NS, JB], BF16, tag="ut", name=f"ut_{J}")
            uts[J] = ut
            for sp in range(J + 1):
                si0 = 2 * sp
                # late big chunks: borrow the (idle-by-then) proj pool for
                # every third score PSUM so exp (Act) has 6 slots of slack
                pool = pp if (J >= 5 and sp % 3 == 2) else pst
                ps_t = pool.tile([P, 2 * JB], F32,
                                 tag="pst" if pool is pst else "pp",
                                 name=f"pst_{J}_{sp}")
                if sp < J:
                    for h in range(2):
                        si = si0 + h
                        for ct in range(2):
                            nc.tensor.matmul(
                                ps_t[:, h * JB:(h + 1) * JB],
                                qkT["k"][:, ct, si * P:(si + 1) * P],
                                qkT["q"][:, ct, jt],
                                start=(ct == 0), stop=(ct == 1),
                            )
                    nc.scalar.activation(ut[:, si0:si0 + 2, :], ps_t[:],
                                         mybir.ActivationFunctionType.Exp)
                else:
                    # diagonal pair: si0 needs all 256 t-cols; si0+1 only
                    # its second 128 (AV q=0 never reads si0+1) -> N=128
                    for ct in range(2):
                        nc.tensor.matmul(
                            ps_t[:, 0:JB],
                            qkT["k"][:, ct, si0 * P:(si0 + 1) * P],
                            qkT["q"][:, ct, jt],
                            start=(ct == 0), stop=(ct == 1),
                        )
                    for ct in range(2):
                        nc.tensor.matmul(
                            ps_t[:, JB:JB + P],
                            qkT["k"][:, ct, (si0 + 1) * P:(si0 + 2) * P],
                            qkT["q"][:, ct, J * JB + P:(J + 1) * JB],
                            start=(ct == 0), stop=(ct == 1),
                        )
                    nc.scalar.activation(ut[:, si0, :], ps_t[:, 0:JB],
                                         mybir.ActivationFunctionType.Exp)
                    nc.scalar.activation(ut[:, si0 + 1, P:JB],
                                         ps_t[:, JB:JB + P],
                                         mybir.ActivationFunctionType.Exp)
                    # defer the DVE mask-mults until after do_v's vg-adds:
                    # the masks only gate AV(J) (emitted a stage later, >3us
                    # of slack) while the vg-adds free pav PSUM slots (tight)
                    pending_masks.append((ut, si0))

        def do_v(tb):
            # V projection for chunk tb's two s-tiles
            for si in (2 * tb, 2 * tb + 1):
                pv = pav.tile([P, C + 1], F32, tag="pav", name=f"pv_{si}")[:, :JB]
                for d in range(ND):
                    nc.tensor.matmul(
                        pv,
                        xtr[:, d, si * P:(si + 1) * P],
                        wr["v"][:, d, :],
                        start=(d == 0), stop=(d == ND - 1),
                    )
                nc.vector.tensor_tensor(vg[:, si, :C], pv, bvb_t[:],
                                        mybir.AluOpType.add)
            flush_masks()

        def do_av(J):
            # pending diag masks must be applied before AV reads ut
            flush_masks()
            ut = uts[J]
            for q in range(2):
                tci = 2 * J + q
                pa = pav.tile([P, C + 1], F32, tag="pav", name=f"pav_{J}_{q}")
                if not (J == NJB - 1 and q == 1):
                    for si in range(tci + 1):
                        nc.tensor.matmul(
                            pa[:],
                            ut[:, si, q * P:(q + 1) * P],
                            vg[:, si, :],
                            start=(si == 0), stop=(si == tci),
                        )
                    recip = outp.tile([P, 1], F32, tag="recip")
                    nc.vector.reciprocal(recip[:], pa[:, C:C + 1])
                    ob = outp.tile([P, C], FP16, tag="ob")
                    nc.vector.tensor_scalar_mul(ob[:], pa[:, :C], recip[:])
                    # Pool/SWDGE path keeps HWDGE + the SP sequencer free
                    eng = nc.sync if tci == NS - 1 else nc.gpsimd
                    eng.dma_start(o[tci * P:(tci + 1) * P, :], ob[:])
                else:
                    # final output tile: column-split accumulation so the
                    # upper half's recip/scale/DMA chain overlaps the lower
                    # half's matmuls, shortening the end-of-kernel chain.
                    # Upper half includes the ones-column (rowsum -> recip).
                    for si in range(tci + 1):
                        nc.tensor.matmul(
                            pa[:, P:],
                            ut[:, si, q * P:(q + 1) * P],
                            vg[:, si, P:],
                            start=(si == 0), stop=(si == tci),
                        )
                    recip = outp.tile([P, 1], F32, tag="recip")
                    nc.vector.reciprocal(recip[:], pa[:, C:C + 1])
                    ob = outp.tile([P, C], FP16, tag="ob")
                    nc.vector.tensor_scalar_mul(ob[:, P:], pa[:, P:C], recip[:])
                    nc.sync.dma_start(o[tci * P:(tci + 1) * P, P:], ob[:, P:])
                    # separate PSUM tile: tile-granular PSUM deps would
                    # otherwise serialize these matmuls behind A's readers
                    pa_b = pav.tile([P, C + 1], F32, tag="pav",
                                    name=f"pav_{J}_{q}b")
                    for si in range(tci + 1):
                        nc.tensor.matmul(
                            pa_b[:, :P],
                            ut[:, si, q * P:(q + 1) * P],
                            vg[:, si, :P],
                            start=(si == 0), stop=(si == tci),
                        )
                    nc.vector.tensor_scalar_mul(ob[:, :P], pa_b[:, :P], recip[:])
                    nc.sync.dma_start(o[tci * P:(tci + 1) * P, :P], ob[:, :P])

        def do_proj(psl):
            pw = psl.stop - psl.start
            for mi, mat in enumerate(("q", "k")):
                for ct in range(2):
                    ps_p = pp.tile([P, 2 * JB], F32, tag="pp",
                                   name=f"pp_{mat}{ct}_{psl.start}")[:, :pw]
                    for d in range(ND):
                        nc.tensor.matmul(
                            ps_p,
                            wr[mat][:, d, ct * P:(ct + 1) * P],
                            xtr[:, d, psl],
                            start=(d == 0), stop=(d == ND - 1),
                        )
                    # bias-add on the Activation engine (per-partition bias):
                    # keeps the DVE queue short so vg-adds aren't delayed.
                    nc.scalar.activation(
                        qkT[mat][:, ct, psl], ps_p,
                        mybir.ActivationFunctionType.Identity,
                        bias=bqk_t[:, 2 * mi + ct: 2 * mi + ct + 1],
                    )

        # ---- startup: d-granular interleaved gate DMAs.  st0 is gated on
        # wq+wk+x[ts0] (1.5MB); the serial DMA pipe paces everything, so the
        # order below matches PE consumption order (q-proj d0-3, q d4-7,
        # k d0-3, k d4-7), with the tiny bias/mask DMAs deferred past the
        # gate and x[ts1]/wv/x[ts23] queued just-in-time behind it. ----
        ts0 = slice(0, JB)
        ts1 = slice(JB, 2 * JB)
        pts23 = slice(2 * JB, 4 * JB)
        xsrc = xT.rearrange("(o p) t -> p o t", p=P)
        wq_src = wq.rearrange("(o p) c -> p o c", p=P)
        wk_src = wk.rearrange("(o p) c -> p o c", p=P)
        wv_src = wv.rearrange("(o p) c -> p o c", p=P)
        nc.sync.dma_start(wr["q"][:, 0:5, :], wq_src[:, 0:5, :])
        nc.sync.dma_start(xtr[:, 0:5, ts0], xsrc[:, 0:5, ts0])
        nc.sync.dma_start(wr["q"][:, 5:8, :], wq_src[:, 5:8, :])
        nc.sync.dma_start(xtr[:, 5:8, ts0], xsrc[:, 5:8, ts0])
        nc.sync.dma_start(xtr[:, 0:4, ts1], xsrc[:, 0:4, ts1])
        nc.sync.dma_start(wr["k"][:, 0:4, :], wk_src[:, 0:4, :])
        nc.sync.dma_start(xtr[:, 4:8, ts1], xsrc[:, 4:8, ts1])
        nc.sync.dma_start(wr["k"][:, 4:8, :], wk_src[:, 4:8, :])
        nc.sync.dma_start(bqk_t[:], bqk)
        nc.sync.dma_start(msk_t[:], msk)
        nc.sync.dma_start(bvb_t[:], bvb)
        nc.sync.dma_start(wr["v"][:], wv_src)
        nc.sync.dma_start(xtr[:, :, pts23], xsrc[:, :, pts23])
        pts45 = slice(4 * JB, 6 * JB)
        pts67 = slice(6 * JB, 8 * JB)
        nc.sync.dma_start(xtr[:, :, pts45], xsrc[:, :, pts45])
        nc.sync.dma_start(xtr[:, :, pts67], xsrc[:, :, pts67])

        # Projections for chunks 0+1 via 4 PSUM accumulators, each with two
        # independent 256-col accumulation regions (ts0 / ts1).  Emission
        # phases are ordered to match the DMA arrival sequence (the per-
        # engine dispatcher follows emission priority with limited
        # lookahead, so order must track arrivals).
        # one PSUM tile per (mat, chunk, ct): PSUM deps are tile-granular,
        # so sharing a tile across chunks would serialize the ts0 bias-adds
        # behind the ts1 matmuls.  q-ts0 lives in pp, k-ts0/k-ts1 fill
        # pst's four slots, q-ts1 borrows the (still idle) pav slots.
        psq0 = [pp.tile([P, 2 * JB], F32, tag="pp", name=f"psq0{c}")[:, :JB]
                for c in range(2)]
        psk0 = [pst.tile([P, 2 * JB], F32, tag="pst", name=f"psk0{c}")[:, :JB]
                for c in range(2)]
        psk1 = [pst.tile([P, 2 * JB], F32, tag="pst", name=f"psk1{c}")[:, :JB]
                for c in range(2)]
        psq1 = [pav.tile([P, C + 1], F32, tag="pav", name=f"psq1{c}")[:, :JB]
                for c in range(2)]
        accs01 = {"q": [psq0, psq1], "k": [psk0, psk1]}

        def proj01_mm(mat, tsi, ds):
            for ct in range(2):
                for d in ds:
                    nc.tensor.matmul(
                        accs01[mat][tsi][ct],
                        wr[mat][:, d, ct * P:(ct + 1) * P],
                        xtr[:, d, tsi * JB:(tsi + 1) * JB],
                        start=(d == 0), stop=(d == ND - 1))

        proj01_mm("q", 0, range(0, 5))
        pad(NPAD1)
        proj01_mm("q", 0, range(5, 8))
        proj01_mm("q", 1, range(0, 4))
        proj01_mm("k", 0, range(0, 4))
        proj01_mm("q", 1, range(4, 8))
        proj01_mm("k", 0, range(4, 8))
        proj01_mm("k", 1, range(0, 4))
        proj01_mm("k", 1, range(4, 8))
        # per-(mat, chunk, ct) DVE bias-adds, in accumulator-stop order
        for mat, mi, tsi in (("q", 0, 0), ("q", 0, 1), ("k", 1, 0),
                             ("k", 1, 1)):
            for ct in range(2):
                nc.vector.tensor_scalar_add(
                    qkT[mat][:, ct, tsi * JB:(tsi + 1) * JB],
                    accs01[mat][tsi][ct],
                    bqk_t[:, 2 * mi + ct: 2 * mi + ct + 1])
        pad(NPAD2)
        # Software-pipelined chunk loop: each AV(t) is emitted one stage
        # after its st(t)/V(t), so the DVE mask/bias chains and Act exps
        # that gate AV's final si-matmuls always have >1.5us of PE runway
        # and never stall the tensor engine.
        do_v(0)
        do_st(0)
        do_v(1)
        do_st(1)
        do_av(0)
        do_proj(pts23)
        do_av(1)
        do_st(2)
        do_v(2)
        do_st(3)
        do_v(3)
        do_av(2)
        do_proj(pts45)
        do_av(3)
        do_st(4)
        do_v(4)
        do_st(5)
        do_v(5)
        do_av(4)
        do_proj(pts67)
        do_av(5)
        do_st(6)
        do_v(6)
        do_st(7)
        do_v(7)
        do_av(6)
        do_av(7)

    nc.compile()
    return nc


_CACHE = {}
LAST_EXEC_TIME_NS = None


def _get_nc():
    if "nc" not in _CACHE:
        _CACHE["nc"] = _build()
    return _CACHE["nc"]


def kernel(x, Wq, bq, Wk, bk, Wv, bv):
    x = np.asarray(x, dtype=np.float32)
    Wq = np.asarray(Wq, dtype=np.float32)
    bq = np.asarray(bq, dtype=np.float32)
    Wk = np.asarray(Wk, dtype=np.float32)
    bk = np.asarray(bk, dtype=np.float32)
    Wv = np.asarray(Wv, dtype=np.float32)
    bv = np.asarray(bv, dtype=np.float32)

    # Fold the 4x head-tiling into the weights: contraction with tile(Kg,4)
    # equals contraction of block-summed Q with Kg.
    wq_s = Wq.reshape(D, 4, C).sum(axis=1, dtype=np.float64).astype(np.float32)
    bq_s = bq.reshape(4, C).sum(axis=0, dtype=np.float64).astype(np.float32)

    bqk = np.stack([bq_s[:P], bq_s[P:], bk[:P], bk[P:]], axis=1).astype(np.float32)
    bvb = np.broadcast_to(bv, (P, C)).astype(np.float32)

    # Diagonal-block causal masks: keep t >= s  <=>  j >= 128*m + p.
    jj = np.arange(JB)[None, None, :]
    pp_ = np.arange(P)[:, None, None]
    mm = np.arange(2)[None, :, None]
    msk = (jj >= P * mm + pp_).astype(ml_dtypes.float8_e4m3)

    shared = {
        "wq": np.ascontiguousarray(wq_s.astype(np.float16)),
        "wk": np.ascontiguousarray(Wk.astype(np.float16)),
        "wv": np.ascontiguousarray(Wv.astype(np.float16)),
        "bqk": np.ascontiguousarray(bqk), "bvb": np.ascontiguousarray(bvb.astype(np.float16)),
        "msk": np.ascontiguousarray(msk),
    }
    in_maps = []
    for b in range(B):
        m = dict(shared)
        m["xT"] = np.ascontiguousarray(x[b].T.astype(np.float16))
        in_maps.append(m)

    nc = _get_nc()
    try:
        res = run_bass_kernel_spmd(nc, in_maps, core_ids=list(range(NCORES)))
    except ModuleNotFoundError:
        # BASS_TRACE=1 requests NTFF profiling, but this container type has
        # no axon NTFF hook (antenv.axon_hooks absent) -- rerun untraced.
        os.environ["BASS_NEVER_TRACE"] = "1"
        res = run_bass_kernel_spmd(nc, in_maps, core_ids=list(range(NCORES)))
    global LAST_EXEC_TIME_NS
    LAST_EXEC_TIME_NS = res.exec_time_ns
    if res.exec_time_ns is not None:
        print(f"HW exec time: {res.exec_time_ns} ns")

    out = np.empty((1, B, T, 4 * C), dtype=np.float32)
    for b in range(B):
        ob = np.asarray(res.results[b]["o"], dtype=np.float32)
        out[0, b] = np.tile(ob, (1, 4))
    return out



# revision 58
# speedup vs baseline: 1.0390x; 1.0016x over previous
"""Causal attention (flattened-head GQA variant) for TRN2, 8 NeuronCores.

Problem structure exploited:
  - K/V are group-projections tiled 4x along the head dim, and the score
    contraction runs over the full flattened 1024 dim.  Algebraically:
        att = Q @ tile(Kg,4)^T = (sum of Q's four 256-col blocks) @ Kg^T
        out = att_sm @ tile(Vg,4) = tile(att_sm @ Vg, 4)
    so the device only computes with 256-wide Qsum/Kg/Vg.
  - Softmax needs no max-subtraction here (logits bounded ~60; exp fits fp32
    comfortably), so scores are computed directly in the transposed layout
    U^T[s,t] = exp(Kg @ Qsum^T) and fed straight into the AV matmul as the
    stationary operand -- no on-device transposes at all.
  - Row sums come from a ones-column appended to Vg (PSUM col 256).
  - Block-causal skipping: s-tiles entirely above the diagonal are never
    computed; diagonal 128x256 blocks are masked with precomputed 0/1 tiles.
  - Fused chunk loop: for each 256-wide t-chunk, DMA x, project Q/K for the
    chunk, project V for its two s-tiles, then compute score block J=chunk
    (causally needs only chunks <= J) and its AV output.  This fills the
    DMA-paced load phase with score/AV compute and keeps the PE saturated.

Schedule notes (tuned against the cost-model timeline sim):
  - The startup is DMA-pipe-bound: gate DMAs (wq, x[ts0], x[ts1], wk) are
    split into d-halves ordered exactly as the single serial DMA pipe
    delivers them, and the chunk-0/1 projections are emitted d-phase by
    d-phase to match (per-engine dispatch follows emission priority).
  - One PSUM tile per (mat, chunk, ct) accumulator: PSUM deps are
    tile-granular, and interleaving two accumulation start/stop groups
    inside one PSUM tile breaks real-hw lowering (keep chains per tile
    contiguous; interleaving chains across different tiles is fine).
  - Warm-up matmuls on a Pool-memset scratch bridge the pipe-fill idle so
    the PE p-state ramp finishes before real work starts.
  - Each AV(t) is emitted a pipeline stage after st(t)/V(t), so the DVE
    mask/bias chains and Act exps that gate AV's last si-matmuls always
    have PE runway.
  - Output tiles are fp16 and leave via the Pool/SWDGE DMA path (keeps
    HWDGE + the SP sequencer free); the final tile is column-split so the
    first half's recip/scale/DMA chain overlaps the second half's matmuls.

Precision: QK path in fp16 (11-bit mantissa, full PE rate, half DMA), scores
accumulated in fp32 PSUM, exp/AV path in bf16 (needs bf16's exponent range:
unnormalized exp values reach ~1e26), output fp16 (absmax ~0.37, step 2e-4).
End-to-end absmax rel error vs the fp32 reference ~7e-3.

Sharding: data-parallel over batch B=8, one batch per core, no collectives.
"""

import os
import numpy as np
import ml_dtypes
from contextlib import ExitStack

import concourse.tile as tile
from concourse import bacc, mybir
from concourse.bass_utils import run_bass_kernel_spmd

B, T, D = 8, 2048, 1024
C = 256          # group width (N_QUERY_GROUPS * HEAD_SIZE)
P = 128
ND = D // P      # 8 contraction tiles for projections
NS = T // P      # 16 s-tiles
JB = 256         # t-chunk width
NJB = T // JB    # 8
NCORES = 8
NWARM = 19       # bridge the DMA pipe-fill so real matmuls start at full clock
NPAD1 = 0        # PE filler between q-proj d0-3 and d4-7 (x arrival gap)
NPAD2 = 0        # PE filler covering the qk bias-add (DVE) latency

F32 = mybir.dt.float32
FP16 = mybir.dt.float16
BF16 = mybir.dt.bfloat16


def _build():
    nc = bacc.Bacc("TRN2", target_bir_lowering=False, debug=False)
    xT = nc.dram_tensor("xT", [D, T], FP16, kind="ExternalInput").ap()
    wq = nc.dram_tensor("wq", [D, C], FP16, kind="ExternalInput").ap()
    wk = nc.dram_tensor("wk", [D, C], FP16, kind="ExternalInput").ap()
    wv = nc.dram_tensor("wv", [D, C], FP16, kind="ExternalInput").ap()
    bqk = nc.dram_tensor("bqk", [P, 4], F32, kind="ExternalInput").ap()
    bvb = nc.dram_tensor("bvb", [P, C], FP16, kind="ExternalInput").ap()
    msk = nc.dram_tensor("msk", [P, 2, JB], mybir.dt.float8e4, kind="ExternalInput").ap()
    o = nc.dram_tensor("o", [T, C], FP16, kind="ExternalOutput").ap()

    with tile.TileContext(nc) as tc, ExitStack() as ctx:
        cst = ctx.enter_context(tc.tile_pool(name="cst", bufs=1))
        big = ctx.enter_context(tc.tile_pool(name="big", bufs=1))
        up = ctx.enter_context(tc.tile_pool(name="up", bufs=3))
        outp = ctx.enter_context(tc.tile_pool(name="outp", bufs=3))
        pp = ctx.enter_context(tc.tile_pool(name="pp", bufs=2, space="PSUM"))
        pst = ctx.enter_context(tc.tile_pool(name="pst", bufs=4, space="PSUM"))
        pav = ctx.enter_context(tc.tile_pool(name="pav", bufs=2, space="PSUM"))

        bqk_t = cst.tile([P, 4], F32, tag="bqk")
        bvb_t = cst.tile([P, C], FP16, tag="bvb")
        msk_t = cst.tile([P, 2, JB], mybir.dt.float8e4, tag="msk")

        wr = {}
        for _n in ("q", "k", "v"):
            wr[_n] = cst.tile([P, ND, C], FP16, tag=f"w{_n}", name=f"wr_{_n}")

        # PE warm-up scratch: memset on the Pool engine, which is idle and
        # issues within ~100ns (DVE's first slot is ~0.8us in), so warm-up
        # matmuls start almost immediately, fill the DMA pipe-fill window,
        # and complete the PE p-state ramp (full clock 3us after first busy).
        wrm = cst.tile([P, C], FP16, tag="wrm")
        nc.gpsimd.memset(wrm[:], 0.0)
        for wi in range(NWARM):
            ps_w = pp.tile([P, 2 * JB], F32, tag="pp", name=f"warm_{wi}")
            nc.tensor.matmul(ps_w[:, :C], wrm[:, :P], wrm[:], start=True, stop=True)

        _padn = [0]

        def pad(n):
            # PE filler matmuls (dep-free) to bridge known DMA-arrival gaps;
            # allocated from pav, which is idle until the first AV.
            for _ in range(n):
                _padn[0] += 1
                ps_f = pav.tile([P, C + 1], F32, tag="pav",
                                name=f"padmm_{_padn[0]}")
                nc.tensor.matmul(ps_f[:, :C], wrm[:, :P], wrm[:],
                                 start=True, stop=True)

        xtr = big.tile([P, ND, T], FP16, tag="xtr")
        qkT = {"q": big.tile([P, 2, T], FP16, tag="qsT", name="qsT"),
               "k": big.tile([P, 2, T], FP16, tag="ksT", name="ksT")}
        vg = big.tile([P, NS, C + 1], BF16, tag="vg")
        nc.vector.memset(vg[:, :, C:C + 1], 8.0)

        uts = {}
        pending_masks = []

        def flush_masks():
            while pending_masks:
                ut, si0 = pending_masks.pop(0)
                nc.vector.tensor_tensor(ut[:, si0, :], ut[:, si0, :],
                                        msk_t[:, 0, :],
                                        mybir.AluOpType.mult)
                nc.vector.tensor_tensor(ut[:, si0 + 1, P:JB],
                                        ut[:, si0 + 1, P:JB],
                                        msk_t[:, 1, P:JB],
                                        mybir.AluOpType.mult)

        def do_st(J):
            # scores^T -> exp for t-block J
            jt = slice(J * JB, (J + 1) * JB)
            ut = up.tile([P, NS, JB], BF16, tag="ut", name=f"ut_{J}")
            uts[J] = ut
            for sp in range(J + 1):
                si0 = 2 * sp
                # late big chunks: borrow the (idle-by-then) proj pool for
                # every third score PSUM so exp (Act) has 6 slots of slack
                pool = pp if (J >= 5 and sp % 3 == 2) else pst
                ps_t = pool.tile([P, 2 * JB], F32,
                                 tag="pst" if pool is pst else "pp",
                                 name=f"pst_{J}_{sp}")
                if sp < J:
                    for h in range(2):
                        si = si0 + h
                        for ct in range(2):
                            nc.tensor.matmul(
                                ps_t[:, h * JB:(h + 1) * JB],
                                qkT["k"][:, ct, si * P:(si + 1) * P],
                                qkT["q"][:, ct, jt],
                                start=(ct == 0), stop=(ct == 1),
                            )
                    nc.scalar.activation(ut[:, si0:si0 + 2, :], ps_t[:],
                                         mybir.ActivationFunctionType.Exp)
                else:
                    # diagonal pair: si0 needs all 256 t-cols; si0+1 only
                    # its second 128 (AV q=0 never reads si0+1) -> N=128
                    for ct in range(2):
                        nc.tensor.matmul(
                            ps_t[:, 0:JB],
                            qkT["k"][:, ct, si0 * P:(si0 + 1) * P],
                            qkT["q"][:, ct, jt],
                            start=(ct == 0), stop=(ct == 1),
                        )
                    for ct in range(2):
                        nc.tensor.matmul(
                            ps_t[:, JB:JB + P],
                            qkT["k"][:, ct, (si0 + 1) * P:(si0 + 2) * P],
                            qkT["q"][:, ct, J * JB + P:(J + 1) * JB],
                            start=(ct == 0), stop=(ct == 1),
                        )
                    nc.scalar.activation(ut[:, si0, :], ps_t[:, 0:JB],
                                         mybir.ActivationFunctionType.Exp)
                    nc.scalar.activation(ut[:, si0 + 1, P:JB],
                                         ps_t[:, JB:JB + P],
                                         mybir.ActivationFunctionType.Exp)
                    # defer the DVE mask-mults until after do_v's vg-adds:
                    # the masks only gate AV(J) (emitted a stage later, >3us
                    # of slack) while the vg-adds free pav PSUM slots (tight)
                    pending_masks.append((ut, si0))

        def do_v(tb):
            # V projection for chunk tb's two s-tiles
            for si in (2 * tb, 2 * tb + 1):
                pv = pav.tile([P, C + 1], F32, tag="pav", name=f"pv_{si}")[:, :JB]
                for d in range(ND):
                    nc.tensor.matmul(
                        pv,
                        xtr[:, d, si * P:(si + 1) * P],
                        wr["v"][:, d, :],
                        start=(d == 0), stop=(d == ND - 1),
                    )
                nc.vector.tensor_tensor(vg[:, si, :C], pv, bvb_t[:],
                                        mybir.AluOpType.add)
            flush_masks()

        def do_av(J):
            # pending diag masks must be applied before AV reads ut
            flush_masks()
            ut = uts[J]
            for q in range(2):
                tci = 2 * J + q
                pa = pav.tile([P, C + 1], F32, tag="pav", name=f"pav_{J}_{q}")
                if not (J == NJB - 1 and q == 1):
                    for si in range(tci + 1):
                        nc.tensor.matmul(
                            pa[:],
                            ut[:, si, q * P:(q + 1) * P],
                            vg[:, si, :],
                            start=(si == 0), stop=(si == tci),
                        )
                    recip = outp.tile([P, 1], F32, tag="recip")
                    nc.vector.reciprocal(recip[:], pa[:, C:C + 1])
                    ob = outp.tile([P, C], FP16, tag="ob")
                    nc.vector.tensor_scalar_mul(ob[:], pa[:, :C], recip[:])
                    # Pool/SWDGE path keeps HWDGE + the SP sequencer free
                    eng = nc.sync if tci == NS - 1 else nc.gpsimd
                    eng.dma_start(o[tci * P:(tci + 1) * P, :], ob[:])
                else:
                    # final output tile: column-split accumulation so the
                    # upper half's recip/scale/DMA chain overlaps the lower
                    # half's matmuls, shortening the end-of-kernel chain.
                    # Upper half includes the ones-column (rowsum -> recip).
                    for si in range(tci + 1):
                        nc.tensor.matmul(
                            pa[:, P:],
                            ut[:, si, q * P:(q + 1) * P],
                            vg[:, si, P:],
                            start=(si == 0), stop=(si == tci),
                        )
                    recip = outp.tile([P, 1], F32, tag="recip")
                    nc.vector.reciprocal(recip[:], pa[:, C:C + 1])
                    ob = outp.tile([P, C], FP16, tag="ob")
                    nc.vector.tensor_scalar_mul(ob[:, P:], pa[:, P:C], recip[:])
                    nc.sync.dma_start(o[tci * P:(tci + 1) * P, P:], ob[:, P:])
                    # separate PSUM tile: tile-granular PSUM deps would
                    # otherwise serialize these matmuls behind A's readers
                    pa_b = pav.tile([P, C + 1], F32, tag="pav",
                                    name=f"pav_{J}_{q}b")
                    for si in range(tci + 1):
                        nc.tensor.matmul(
                            pa_b[:, :P],
                            ut[:, si, q * P:(q + 1) * P],
                            vg[:, si, :P],
                            start=(si == 0), stop=(si == tci),
                        )
                    nc.vector.tensor_scalar_mul(ob[:, :P], pa_b[:, :P], recip[:])
                    nc.sync.dma_start(o[tci * P:(tci + 1) * P, :P], ob[:, :P])

        def do_proj(psl):
            pw = psl.stop - psl.start
            for mi, mat in enumerate(("q", "k")):
                for ct in range(2):
                    ps_p = pp.tile([P, 2 * JB], F32, tag="pp",
                                   name=f"pp_{mat}{ct}_{psl.start}")[:, :pw]
                    for d in range(ND):
                        nc.tensor.matmul(
                            ps_p,
                            wr[mat][:, d, ct * P:(ct + 1) * P],
                            xtr[:, d, psl],
                            start=(d == 0), stop=(d == ND - 1),
                        )
                    # bias-add on the Activation engine (per-partition bias):
                    # keeps the DVE queue short so vg-adds aren't delayed.
                    nc.scalar.activation(
                        qkT[mat][:, ct, psl], ps_p,
                        mybir.ActivationFunctionType.Identity,
                        bias=bqk_t[:, 2 * mi + ct: 2 * mi + ct + 1],
                    )

        # ---- startup: d-granular interleaved gate DMAs.  st0 is gated on
        # wq+wk+x[ts0] (1.5MB); the serial DMA pipe paces everything, so the
        # order below matches PE consumption order (q-proj d0-3, q d4-7,
        # k d0-3, k d4-7), with the tiny bias/mask DMAs deferred past the
        # gate and x[ts1]/wv/x[ts23] queued just-in-time behind it. ----
        ts0 = slice(0, JB)
        ts1 = slice(JB, 2 * JB)
        pts23 = slice(2 * JB, 4 * JB)
        xsrc = xT.rearrange("(o p) t -> p o t", p=P)
        wq_src = wq.rearrange("(o p) c -> p o c", p=P)
        wk_src = wk.rearrange("(o p) c -> p o c", p=P)
        wv_src = wv.rearrange("(o p) c -> p o c", p=P)
        nc.sync.dma_start(wr["q"][:, 0:5, :], wq_src[:, 0:5, :])
        nc.sync.dma_start(xtr[:, 0:5, ts0], xsrc[:, 0:5, ts0])
        nc.sync.dma_start(wr["q"][:, 5:8, :], wq_src[:, 5:8, :])
        nc.sync.dma_start(xtr[:, 5:8, ts0], xsrc[:, 5:8, ts0])
        nc.sync.dma_start(xtr[:, 0:4, ts1], xsrc[:, 0:4, ts1])
        nc.sync.dma_start(wr["k"][:, 0:4, :], wk_src[:, 0:4, :])
        nc.sync.dma_start(xtr[:, 4:8, ts1], xsrc[:, 4:8, ts1])
        nc.sync.dma_start(wr["k"][:, 4:8, :], wk_src[:, 4:8, :])
        nc.sync.dma_start(bqk_t[:], bqk)
        nc.sync.dma_start(msk_t[:], msk)
        nc.sync.dma_start(bvb_t[:], bvb)
        nc.sync.dma_start(wr["v"][:], wv_src)
        nc.sync.dma_start(xtr[:, :, pts23], xsrc[:, :, pts23])
        pts45 = slice(4 * JB, 6 * JB)
        pts67 = slice(6 * JB, 8 * JB)
        nc.sync.dma_start(xtr[:, :, pts45], xsrc[:, :, pts45])
        nc.sync.dma_start(xtr[:, :, pts67], xsrc[:, :, pts67])

        # Projections for chunks 0+1 via 4 PSUM accumulators, each with two
        # independent 256-col accumulation regions (ts0 / ts1).  Emission
        # phases are ordered to match the DMA arrival sequence (the per-
        # engine dispatcher follows emission priority with limited
        # lookahead, so order must track arrivals).
        # one PSUM tile per (mat, chunk, ct): PSUM deps are tile-granular,
        # so sharing a tile across chunks would serialize the ts0 bias-adds
        # behind the ts1 matmuls.  q-ts0 lives in pp, k-ts0/k-ts1 fill
        # pst's four slots, q-ts1 borrows the (still idle) pav slots.
        psq0 = [pp.tile([P, 2 * JB], F32, tag="pp", name=f"psq0{c}")[:, :JB]
                for c in range(2)]
        psk0 = [pst.tile([P, 2 * JB], F32, tag="pst", name=f"psk0{c}")[:, :JB]
                for c in range(2)]
        psk1 = [pst.tile([P, 2 * JB], F32, tag="pst", name=f"psk1{c}")[:, :JB]
                for c in range(2)]
        psq1 = [pav.tile([P, C + 1], F32, tag="pav", name=f"psq1{c}")[:, :JB]
                for c in range(2)]
        accs01 = {"q": [psq0, psq1], "k": [psk0, psk1]}

        def proj01_mm(mat, tsi, ds):
            for ct in range(2):
                for d in ds:
                    nc.tensor.matmul(
                        accs01[mat][tsi][ct],
                        wr[mat][:, d, ct * P:(ct + 1) * P],
                        xtr[:, d, tsi * JB:(tsi + 1) * JB],
                        start=(d == 0), stop=(d == ND - 1))

        proj01_mm("q", 0, range(0, 5))
        pad(NPAD1)
        proj01_mm("q", 0, range(5, 8))
        proj01_mm("q", 1, range(0, 4))
        proj01_mm("k", 0, range(0, 4))
        proj01_mm("q", 1, range(4, 8))
        proj01_mm("k", 0, range(4, 8))
        proj01_mm("k", 1, range(0, 4))
        proj01_mm("k", 1, range(4, 8))
        # per-(mat, chunk, ct) DVE bias-adds, in accumulator-stop order
        for mat, mi, tsi in (("q", 0, 0), ("q", 0, 1), ("k", 1, 0),
                             ("k", 1, 1)):
            for ct in range(2):
                nc.vector.tensor_scalar_add(
                    qkT[mat][:, ct, tsi * JB:(tsi + 1) * JB],
                    accs01[mat][tsi][ct],
                    bqk_t[:, 2 * mi + ct: 2 * mi + ct + 1])
        pad(NPAD2)
        # Software-pipelined chunk loop: each AV(t) is emitted one stage
        # after its st(t)/V(t), so the DVE mask/bias chains and Act exps
        # that gate AV's final si-matmuls always have >1.5us of PE runway
        # and never stall the tensor engine.
        do_v(0)
        do_st(0)
        do_v(1)
        do_st(1)
        do_av(0)
        do_proj(pts23)
        do_av(1)
        do_st(2)
        do_v(2)
        do_st(3)
        do_v(3)
        do_av(2)
        do_proj(pts45)
        do_av(3)
        do_st(4)
        do_v(4)
        do_st(5)
        do_v(5)
        do_av(4)
        do_proj(pts67)
        do_av(5)
        do_st(6)
        do_v(6)
        do_st(7)
        do_v(7)
        do_av(6)
        do_av(7)

    nc.compile()
    return nc


_CACHE = {}
LAST_EXEC_TIME_NS = None


def _get_nc():
    if "nc" not in _CACHE:
        _CACHE["nc"] = _build()
    return _CACHE["nc"]


def kernel(x, Wq, bq, Wk, bk, Wv, bv):
    x = np.asarray(x, dtype=np.float32)
    Wq = np.asarray(Wq, dtype=np.float32)
    bq = np.asarray(bq, dtype=np.float32)
    Wk = np.asarray(Wk, dtype=np.float32)
    bk = np.asarray(bk, dtype=np.float32)
    Wv = np.asarray(Wv, dtype=np.float32)
    bv = np.asarray(bv, dtype=np.float32)

    # Fold the 4x head-tiling into the weights: contraction with tile(Kg,4)
    # equals contraction of block-summed Q with Kg.
    wq_s = Wq.reshape(D, 4, C).sum(axis=1, dtype=np.float64).astype(np.float32)
    bq_s = bq.reshape(4, C).sum(axis=0, dtype=np.float64).astype(np.float32)

    bqk = np.stack([bq_s[:P], bq_s[P:], bk[:P], bk[P:]], axis=1).astype(np.float32)
    bvb = np.broadcast_to(bv, (P, C)).astype(np.float32)

    # Diagonal-block causal masks: keep t >= s  <=>  j >= 128*m + p.
    jj = np.arange(JB)[None, None, :]
    pp_ = np.arange(P)[:, None, None]
    mm = np.arange(2)[None, :, None]
    msk = (jj >= P * mm + pp_).astype(ml_dtypes.float8_e4m3)

    shared = {
        "wq": np.ascontiguousarray(wq_s.astype(np.float16)),
        "wk": np.ascontiguousarray(Wk.astype(np.float16)),
        "wv": np.ascontiguousarray(Wv.astype(np.float16)),
        "bqk": np.ascontiguousarray(bqk), "bvb": np.ascontiguousarray(bvb.astype(np.float16)),
        "msk": np.ascontiguousarray(msk),
    }
    in_maps = []
    for b in range(B):
        m = dict(shared)
        m["xT"] = np.ascontiguousarray(x[b].T.astype(np.float16))
        in_maps.append(m)

    nc = _get_nc()
    try:
        res = run_bass_kernel_spmd(nc, in_maps, core_ids=list(range(NCORES)))
    except ModuleNotFoundError:
        # BASS_TRACE=1 requests NTFF profiling, but this container type has
        # no axon NTFF hook (antenv.axon_hooks absent) -- rerun untraced.
        os.environ["BASS_NEVER_TRACE"] = "1"
        res = run_bass_kernel_spmd(nc, in_maps, core_ids=list(range(NCORES)))
    global LAST_EXEC_TIME_NS
    LAST_EXEC_TIME_NS = res.exec_time_ns
    if res.exec_time_ns is not None:
        print(f"HW exec time: {res.exec_time_ns} ns")

    out = np.empty((1, B, T, 4 * C), dtype=np.float32)
    for b in range(B):
        ob = np.asarray(res.results[b]["o"], dtype=np.float32)
        out[0, b] = np.tile(ob, (1, 4))
    return out

